# revision 18
# baseline (speedup 1.0000x reference)
"""Trainium2 Bass kernel for nn_DHSRNN (dendritic hierarchical spiking RNN).

Strategy: 8-way tensor-parallel over the HID*BRANCH=4096 dendritic feature dim
(512 feats / 128 hidden neurons per core), full batch (128) kept on every core
as the matmul free dim. Recurrent spikes are exchanged once per timestep with
an AllGather of each core's (128 hid x 128 batch) bf16 spike tile.

Math restructuring (all exact, validated vs the reference semantics):
 - all biases eliminated from the loop by state shifts:
     e = d - b_dense      (per-branch dendritic state)
     f = mem - c,  c_h = sum_j b_dense[4h+j]   (soma state)
   spike condition becomes a per-partition threshold f > VTH - c, and the
   readout bias contribution is added back in closed form on the host.
 - soft reset folded into a single fused op via z = f - (VTH/alpha)*spk,
   so f_t = alpha*z_{t-1} + drive_t.
 - (1-beta) folded into the dense weights; (1-alpha) and the branch-sum
   folded into a second "membrane" weight set (wmem) and a sparse
   pattern matrix (pat) applied to e, so the soma drive comes straight
   out of PSUM with no branch reduction on the vector engine.
 - input drive (x @ Wx') for all 200 steps precomputed on-device in a
   phase-B batched matmul, streamed back per-step during the loop.

Layout: features/hidden on partitions, batch on the free dim, so every decay
constant is a per-partition scalar for fused scalar_tensor_tensor ops.
"""
import sys
import os

sys.path.insert(0, "/opt/trn_rl_repo")

import numpy as np
import ml_dtypes

IN_DIM, HID, OUT, BRANCH = 512, 1024, 256, 4
B, T = 128, 200
VTH, WARMUP = 1.0, 10
N_CORES = 8
FPC = HID * BRANCH // N_CORES   # 512 feats per core
HPC = HID // N_CORES            # 128 hid per core
NPIECE = T * B // 512           # 50 phase-B column pieces

bf16 = ml_dtypes.bfloat16

_PROG_CACHE = {}


def _sigmoid(x):
    return (1.0 / (1.0 + np.exp(-np.asarray(x, np.float64)))).astype(np.float32)


def build_program():
    """Build the SPMD Bass program (identical on all cores; per-core behavior
    comes only from per-core input data)."""
    from concourse import bacc, tile, mybir

    nc = bacc.Bacc("TRN2", target_bir_lowering=False, debug=False,
                   num_devices=N_CORES)
    f32 = mybir.dt.float32
    b16 = mybir.dt.bfloat16

    # ---- I/O ----
    whT_in = nc.dram_tensor("whT_in", [HID, FPC], b16, kind="ExternalInput").ap()
    wxT_in = nc.dram_tensor("wxT_in", [IN_DIM, FPC], b16, kind="ExternalInput").ap()
    wmemT_in = nc.dram_tensor("wmemT_in", [HID, HPC], b16, kind="ExternalInput").ap()
    wxmemT_in = nc.dram_tensor("wxmemT_in", [IN_DIM, HPC], b16, kind="ExternalInput").ap()
    patT_in = nc.dram_tensor("patT_in", [FPC, HPC], f32, kind="ExternalInput").ap()
    wroT_in = nc.dram_tensor("wroT_in", [HPC, OUT], b16, kind="ExternalInput").ap()
    xT_in = nc.dram_tensor("xT_in", [IN_DIM, T * B], b16, kind="ExternalInput").ap()
    einit_in = nc.dram_tensor("einit_in", [FPC, B], f32, kind="ExternalInput").ap()
    zinit_in = nc.dram_tensor("zinit_in", [HPC, B], f32, kind="ExternalInput").ap()
    alpha_in = nc.dram_tensor("alpha_in", [HPC, 1], f32, kind="ExternalInput").ap()
    nvoa_in = nc.dram_tensor("nvoa_in", [HPC, 1], f32, kind="ExternalInput").ap()
    thr_in = nc.dram_tensor("thr_in", [HPC, 1], f32, kind="ExternalInput").ap()
    beta_in = nc.dram_tensor("beta_in", [HPC, 4], f32, kind="ExternalInput").ap()
    alo_in = nc.dram_tensor("alo_in", [HPC, 2], f32, kind="ExternalInput").ap()
    ident_in = nc.dram_tensor("ident_in", [HPC, HPC], b16, kind="ExternalInput").ap()
    adiag_in = nc.dram_tensor("adiag_in", [HPC, HPC], f32, kind="ExternalInput").ap()

    acc_out = nc.dram_tensor("acc_out", [HPC, OUT], f32, kind="ExternalOutput").ap()
    junk_out = nc.dram_tensor("junk_out", [HPC, 512], f32, kind="ExternalOutput").ap()

    KT = HID // HPC       # 8 hid k-chunks
    KX = IN_DIM // HPC    # 4 input k-chunks
    MT = FPC // HPC       # 4 feat m-chunks
    Add = mybir.AluOpType.add
    Mult = mybir.AluOpType.mult
    IsGt = mybir.AluOpType.is_gt
    Bypass = mybir.AluOpType.bypass

    with tile.TileContext(nc) as tc:
        with tc.tile_pool(name="consts", bufs=1) as cpool, \
             tc.tile_pool(name="state", bufs=1) as spool, \
             tc.tile_pool(name="dramw", bufs=1, space="DRAM") as dpool:

            # ---- resident constants in SBUF ----
            whT_sb = cpool.tile([HPC, KT * FPC], b16)      # (128, 8*512)
            for k in range(KT):
                nc.sync.dma_start(whT_sb[:, k * FPC:(k + 1) * FPC],
                                  whT_in[k * HPC:(k + 1) * HPC, :])
            wmemT_sb = cpool.tile([HPC, KT * HPC], b16)    # (128, 8*128)
            for k in range(KT):
                nc.sync.dma_start(wmemT_sb[:, k * HPC:(k + 1) * HPC],
                                  wmemT_in[k * HPC:(k + 1) * HPC, :])
            wxT_sb = cpool.tile([HPC, KX * FPC], b16)      # (128, 4*512)
            for k in range(KX):
                nc.sync.dma_start(wxT_sb[:, k * FPC:(k + 1) * FPC],
                                  wxT_in[k * HPC:(k + 1) * HPC, :])
            wxmemT_sb = cpool.tile([HPC, KX * HPC], b16)   # (128, 4*128)
            for k in range(KX):
                nc.sync.dma_start(wxmemT_sb[:, k * HPC:(k + 1) * HPC],
                                  wxmemT_in[k * HPC:(k + 1) * HPC, :])
            patT_sb = cpool.tile([HPC, MT * HPC], f32)     # (128, 4*128)
            for k in range(MT):
                nc.sync.dma_start(patT_sb[:, k * HPC:(k + 1) * HPC],
                                  patT_in[k * HPC:(k + 1) * HPC, :])
            wroT_sb = cpool.tile([HPC, OUT], b16)
            nc.sync.dma_start(wroT_sb[:], wroT_in[:])
            ident_sb = cpool.tile([HPC, HPC], b16)
            nc.sync.dma_start(ident_sb[:], ident_in[:])
            adiag_sb = cpool.tile([HPC, HPC], f32)
            nc.sync.dma_start(adiag_sb[:], adiag_in[:])
            alpha_sb = cpool.tile([HPC, 1], f32)
            nc.sync.dma_start(alpha_sb[:], alpha_in[:])
            nvoa_sb = cpool.tile([HPC, 1], f32)
            nc.sync.dma_start(nvoa_sb[:], nvoa_in[:])
            thr_sb = cpool.tile([HPC, 1], f32)
            nc.sync.dma_start(thr_sb[:], thr_in[:])
            beta_sb = cpool.tile([HPC, 4], f32)
            nc.sync.dma_start(beta_sb[:], beta_in[:])
            alo_sb = cpool.tile([HPC, 2], f32)
            nc.sync.dma_start(alo_sb[:], alo_in[:])

            # ---- persistent state ----
            e_sb = spool.tile([HPC, FPC], f32)             # (128, 512): e chunks
            for k in range(MT):
                nc.sync.dma_start(e_sb[:, k * HPC:(k + 1) * HPC],
                                  einit_in[k * HPC:(k + 1) * HPC, :])
            z_sb = spool.tile([HPC, B], f32)
            nc.sync.dma_start(z_sb[:], zinit_in[:])
            g_sb = spool.tile([HPC, OUT], f32)
            nc.vector.memset(g_sb[:], 0.0)
            acc_sb = spool.tile([HPC, OUT], f32)
            nc.vector.memset(acc_sb[:], 0.0)

            # ---- internal DRAM ----
            xdrive_dram = dpool.tile([MT, T, HPC, B], b16)
            xmem_dram = dpool.tile([T, HPC, B], b16)

            # ================= Phase B: x-drive precompute =================
            with tc.tile_pool(name="pbx", bufs=3) as pbx, \
                 tc.tile_pool(name="pbo", bufs=4) as pbo, \
                 tc.tile_pool(name="pbp", bufs=2, space="PSUM") as pbp:
                for p in range(NPIECE):
                    cs = p * 512
                    xtile = pbx.tile([HPC, KX * 512], b16)
                    nc.sync.dma_start(
                        xtile[:].rearrange("p (k c) -> p k c", k=KX),
                        xT_in[:, cs:cs + 512].rearrange("(k p) c -> p k c", k=KX))
                    for m in range(MT):
                        xdp = pbp.tile([HPC, 512], f32, tag="xdp")
                        for k in range(KX):
                            nc.tensor.matmul(
                                xdp[:],
                                lhsT=wxT_sb[:, k * FPC + m * HPC:k * FPC + (m + 1) * HPC],
                                rhs=xtile[:, k * 512:(k + 1) * 512],
                                start=(k == 0), stop=(k == KX - 1))
                        xdo = pbo.tile([HPC, 512], b16, tag="xdo")
                        nc.vector.tensor_copy(xdo[:], xdp[:])
                        nc.scalar.dma_start(
                            xdrive_dram[m, 4 * p:4 * p + 4].rearrange(
                                "tl p b -> p tl b"),
                            xdo[:].rearrange("p (tl b) -> p tl b", tl=4))
                    xmp = pbp.tile([HPC, 512], f32, tag="xmp")
                    for k in range(KX):
                        nc.tensor.matmul(
                            xmp[:],
                            lhsT=wxmemT_sb[:, k * HPC:(k + 1) * HPC],
                            rhs=xtile[:, k * 512:(k + 1) * 512],
                            start=(k == 0), stop=(k == KX - 1))
                    xmo = pbo.tile([HPC, 512], b16, tag="xmo")
                    nc.scalar.copy(xmo[:], xmp[:])
                    nc.gpsimd.dma_start(
                        xmem_dram[4 * p:4 * p + 4].rearrange("tl p b -> p tl b"),
                        xmo[:].rearrange("p (tl b) -> p tl b", tl=4))

            # ================= Phase C: recurrent loop =================
            NFILL = 20
            with tc.tile_pool(name="lio", bufs=3) as lio, \
                 tc.tile_pool(name="lgt", bufs=2) as lgt, \
                 tc.tile_pool(name="lfs", bufs=2) as lfs, \
                 tc.tile_pool(name="ldr", bufs=2, space="DRAM") as ldr, \
                 tc.tile_pool(name="vp", bufs=2, space="PSUM") as vpp, \
                 tc.tile_pool(name="dp", bufs=2, space="PSUM") as dpp, \
                 tc.tile_pool(name="rp", bufs=2, space="PSUM") as rpp, \
                 tc.tile_pool(name="jp", bufs=1, space="PSUM") as jpp:

                junk = jpp.tile([HPC, 512], f32, tag="junk")

                def dp_partials(dp, xm, first_stop):
                    """Gather-independent part of the soma drive: pattern on e,
                    alpha*z leak, x-drive inject."""
                    for k in range(MT):
                        nc.tensor.matmul(
                            dp[:],
                            lhsT=patT_sb[:, k * HPC:(k + 1) * HPC],
                            rhs=e_sb[:, k * HPC:(k + 1) * HPC],
                            start=(k == 0), stop=False)
                    nc.tensor.matmul(dp[:], lhsT=adiag_sb[:], rhs=z_sb[:],
                                     start=False, stop=False)
                    nc.tensor.matmul(dp[:], lhsT=ident_sb[:], rhs=xm[:],
                                     start=False, stop=first_stop)

                def fetch_inputs(t):
                    xdr = lio.tile([HPC, FPC], b16, tag="xdr")
                    nc.scalar.dma_start(
                        xdr[:].rearrange("p (m b) -> p m b", m=MT),
                        xdrive_dram[:, t].rearrange("m p b -> p m b"))
                    xm = lio.tile([HPC, B], b16, tag="xm")
                    nc.scalar.dma_start(xm[:], xmem_dram[t])
                    return xdr, xm

                gt_prev = None   # gathered spikes of step t-1 in SBUF
                xdr, xm = fetch_inputs(0)
                dp = dpp.tile([HPC, B], f32, tag="dp")
                dp_partials(dp, xm, True)   # t=0 group ends here (no gather)
                for t in range(T):
                    if t > 0:
                        for k in range(KT):
                            nc.tensor.matmul(
                                dp[:],
                                lhsT=wmemT_sb[:, k * HPC:(k + 1) * HPC],
                                rhs=gt_prev[:, k * B:(k + 1) * B],
                                start=False, stop=(k == KT - 1))

                    # ---- spike straight off PSUM, then z ----
                    spk = lfs.tile([HPC, B], b16, tag="spk")
                    nc.vector.tensor_scalar(spk[:], dp[:], thr_sb[:], None,
                                            op0=IsGt)
                    # bounce + all-gather (spikes of step t)
                    if t < T - 1:
                        spkb = ldr.tile([HPC, B], b16, tag="spkb")
                        gout = ldr.tile([HID, B], b16, tag="gout",
                                        addr_space="Shared")
                        nc.sync.dma_start(spkb[:], spk[:])
                        nc.gpsimd.collective_compute(
                            "AllGather", Bypass,
                            ins=[spkb.opt()], outs=[gout.opt()],
                            replica_groups=[list(range(N_CORES))])
                        gt = lgt.tile([HPC, KT * B], b16, tag="gt")
                        for (k0, k1), eng in zip(((0, 3), (3, 6), (6, 8)),
                                                 (nc.sync, nc.scalar,
                                                  nc.gpsimd)):
                            nk = k1 - k0
                            eng.dma_start(
                                gt[:, k0 * B:k1 * B].rearrange(
                                    "p (k b) -> p k b", k=nk),
                                gout[k0 * HPC:k1 * HPC, :].rearrange(
                                    "(k p) b -> p k b", k=nk))
                    nc.vector.scalar_tensor_tensor(
                        z_sb[:], in0=spk[:], scalar=nvoa_sb[:], in1=dp[:],
                        op0=Mult, op1=Add)

                    # ---- dendritic drive PSUM + e update ----
                    vp = vpp.tile([HPC, FPC], f32, tag="vp")
                    for m in range(MT):
                        nc.tensor.matmul(
                            vp[:, m * HPC:(m + 1) * HPC],
                            lhsT=ident_sb[:],
                            rhs=xdr[:, m * HPC:(m + 1) * HPC],
                            start=True, stop=(t == 0))
                    if t > 0:
                        for m in range(MT):
                            for k in range(KT):
                                nc.tensor.matmul(
                                    vp[:, m * HPC:(m + 1) * HPC],
                                    lhsT=whT_sb[:, k * FPC + m * HPC:
                                                k * FPC + (m + 1) * HPC],
                                    rhs=gt_prev[:, k * B:(k + 1) * B],
                                    start=False, stop=(k == KT - 1))
                    for m in range(MT):
                        nc.vector.scalar_tensor_tensor(
                            e_sb[:, m * HPC:(m + 1) * HPC],
                            in0=e_sb[:, m * HPC:(m + 1) * HPC],
                            scalar=beta_sb[:, m:m + 1],
                            in1=vp[:, m * HPC:(m + 1) * HPC],
                            op0=Mult, op1=Add)

                    # ---- pre-start next step's gather-independent soma MMs
                    # (they run inside the all-gather window) ----
                    if t < T - 1:
                        xdr, xm = fetch_inputs(t + 1)
                        dp = dpp.tile([HPC, B], f32, tag="dp")
                        dp_partials(dp, xm, False)

                    # ---- readout ----
                    rp = rpp.tile([HPC, OUT], f32, tag="rp")
                    for mo in range(2):
                        nc.tensor.matmul(
                            rp[:, mo * HPC:(mo + 1) * HPC],
                            lhsT=wroT_sb[:, mo * HPC:(mo + 1) * HPC],
                            rhs=spk[:], start=True, stop=True)
                    for mo in range(2):
                        nc.vector.scalar_tensor_tensor(
                            g_sb[:, mo * HPC:(mo + 1) * HPC],
                            in0=g_sb[:, mo * HPC:(mo + 1) * HPC],
                            scalar=alo_sb[:, mo:mo + 1],
                            in1=rp[:, mo * HPC:(mo + 1) * HPC],
                            op0=Mult, op1=Add)
                    if t >= WARMUP:
                        nc.vector.tensor_add(acc_sb[:], acc_sb[:], g_sb[:])

                    # ---- HAM-warmth fillers: keep PE busy through the
                    # all-gather window (results written once at the end) ----
                    if 0 < t < T - 1:
                        for j in range(NFILL):
                            nc.tensor.matmul(
                                junk[:, :256],
                                lhsT=wmemT_sb[:, (j % KT) * HPC:
                                              (j % KT + 1) * HPC],
                                rhs=gt_prev[:, (j % 4) * 256:
                                            (j % 4) * 256 + 256],
                                start=True, stop=True,
                                skip_group_check=True)

                    if t < T - 1:
                        gt_prev = gt

                junk_sb = spool.tile([HPC, 512], f32)
                nc.vector.tensor_copy(junk_sb[:], junk[:])
                nc.sync.dma_start(junk_out[:], junk_sb[:])

            nc.sync.dma_start(acc_out[:], acc_sb[:])

    nc.finalize()
    return nc


def _prep_inputs(x, W_dense, b_dense, mask, tau_n, tau_m, W_ro, b_ro, tau_m_ro):
    x = np.asarray(x, np.float32)
    eff_W = np.asarray(W_dense, np.float32) * np.asarray(mask, np.float32)
    b_dense = np.asarray(b_dense, np.float32)
    beta_f = _sigmoid(tau_n).reshape(-1)         # (4096,)
    alpha = _sigmoid(tau_m)                      # (1024,)
    alpha_o = _sigmoid(tau_m_ro)                 # (256,)
    W_ro = np.asarray(W_ro, np.float32)
    b_ro = np.asarray(b_ro, np.float32)

    Wx = eff_W[:, :IN_DIM]
    Wh = eff_W[:, IN_DIM:]
    xT = np.ascontiguousarray(
        x.transpose(2, 1, 0).reshape(IN_DIM, T * B)).astype(bf16)

    in_maps = []
    for c in range(N_CORES):
        fs = slice(c * FPC, (c + 1) * FPC)
        hs = slice(c * HPC, (c + 1) * HPC)
        ombeta = 1.0 - beta_f[fs]
        omal_h = 1.0 - alpha[hs]
        whT = np.ascontiguousarray((Wh[fs, :] * ombeta[:, None]).T)
        wxT = np.ascontiguousarray((Wx[fs, :] * ombeta[:, None]).T)
        wmem = (Wh[fs, :] * ombeta[:, None]).reshape(HPC, BRANCH, HID).sum(1) \
            * omal_h[:, None]
        wxmem = (Wx[fs, :] * ombeta[:, None]).reshape(HPC, BRANCH, IN_DIM).sum(1) \
            * omal_h[:, None]
        patT = np.zeros((FPC, HPC), np.float32)
        fl = np.arange(FPC)
        patT[fl, fl // 4] = omal_h[fl // 4] * beta_f[fs][fl]
        wroT = np.ascontiguousarray((W_ro[:, hs] * (1.0 - alpha_o)[:, None]).T)
        c_h = b_dense[fs].reshape(HPC, BRANCH).sum(1)
        in_maps.append({
            "whT_in": whT.astype(bf16),
            "wxT_in": wxT.astype(bf16),
            "wmemT_in": np.ascontiguousarray(wmem.T).astype(bf16),
            "wxmemT_in": np.ascontiguousarray(wxmem.T).astype(bf16),
            "patT_in": patT,
            "wroT_in": wroT.astype(bf16),
            "xT_in": xT,
            "einit_in": np.ascontiguousarray(
                np.repeat(-b_dense[fs][:, None], B, 1)).astype(np.float32),
            "zinit_in": np.ascontiguousarray(
                np.repeat(-c_h[:, None], B, 1)).astype(np.float32),
            "alpha_in": alpha[hs].reshape(HPC, 1).copy(),
            "nvoa_in": (-VTH / alpha[hs]).reshape(HPC, 1).astype(np.float32),
            "thr_in": (VTH - c_h).reshape(HPC, 1).astype(np.float32),
            "beta_in": np.ascontiguousarray(
                beta_f[fs].reshape(4, HPC).T).copy(),
            "alo_in": np.ascontiguousarray(
                alpha_o.reshape(2, HPC).T).copy(),
            "ident_in": np.eye(HPC, dtype=np.float32).astype(bf16),
            "adiag_in": np.diag(alpha[hs]).astype(np.float32),
        })

    tt = np.arange(WARMUP, T)
    bias_term = (b_ro.astype(np.float64)
                 * (1.0 - (np.asarray(alpha_o, np.float64)[None, :]
                           ** (tt[:, None] + 1)).mean(0))).astype(np.float32)
    return in_maps, bias_term


def run_kernel(trace=False, **inputs):
    from concourse import bass_utils

    in_maps, bias_term = _prep_inputs(**inputs)
    if "prog" not in _PROG_CACHE:
        _PROG_CACHE["prog"] = build_program()
    nc = _PROG_CACHE["prog"]
    res = bass_utils.run_bass_kernel_spmd(
        nc, in_maps, core_ids=list(range(N_CORES)), trace=trace)

    total = np.zeros((HPC, OUT), np.float32)
    for c in range(N_CORES):
        total += res.results[c]["acc_out"]
    # acc[p, mo*128 + b_... ] layout: [p, mo*B + b] = channel mo*128+p, batch b
    part = total.reshape(HPC, 2, B).transpose(2, 1, 0).reshape(B, OUT)
    out = part / (T - WARMUP) + bias_term[None, :]
    return out.astype(np.float32), res


def kernel(**inputs):
    out, _ = run_kernel(trace=False, **inputs)
    return out
